# revision 8
# baseline (speedup 1.0000x reference)
"""GQA kernel for Trainium2, 8 NeuronCores.

Sharding: core c = (b, g) with b = c // 4 (batch), g = c % 4 (KV group).
Each core computes, for its batch b and group g (4 query heads, 1 KV head):
  qT[d, t] for the 4 heads, kT[d, t], v[t, d] projections (contraction over EMB,
  inputs pre-packed on host so EMB lands on SBUF partitions),
  causal flash-style attention in [k-part, q-free] score layout,
  and the partial output projection  partial_g = (attn out) @ Wp[:, g cols].T.
Host gathers: y[b] = sum_g partial[b, g] + bp.

All matmuls run in bf16 (fp32 PSUM accumulation); host pre-casts inputs.
Inputs are host-packed to the exact SBUF layout [128, free] so each tensor
loads with one contiguous DMA (dma_start issue cost dominates chunked loads).
Causal structure: scores for the diagonal 128-row k-blocks are trimmed to the
q-columns that can attend; only the first 128-col band of each diagonal block
needs an elementwise triangular mask.
"""

import numpy as np
import ml_dtypes

T = 2048
EMB = 2048
HD = 128
GS = 4          # query heads per core (per KV group)
NE = EMB // 128 # 16 contraction chunks
NT = T // 128   # 16 row tiles
NQP = T // 512  # 4 q passes of 512
SCALE = float(HD) ** -0.5

_BF16 = ml_dtypes.bfloat16
_PROGRAM = None


def _build_program():
    import concourse.bass as bass
    import concourse.tile as tile
    from concourse import bacc, mybir
    from concourse.masks import make_identity

    f32 = mybir.dt.float32
    bf16 = mybir.dt.bfloat16

    nc = bacc.Bacc("TRN2", target_bir_lowering=False, debug=False)

    # all inputs host-packed to [128 partitions, free] SBUF layout
    xT_d = nc.dram_tensor("xTp", [128, NE * T], bf16, kind="ExternalInput")
    wq_d = nc.dram_tensor("wqp", [128, NE * GS * HD], bf16, kind="ExternalInput")
    wk_d = nc.dram_tensor("wkp", [128, NE * HD], bf16, kind="ExternalInput")
    wv_d = nc.dram_tensor("wvp", [128, NE * HD], bf16, kind="ExternalInput")
    wp_d = nc.dram_tensor("wpp", [128, GS * EMB], bf16, kind="ExternalInput")
    out_d = nc.dram_tensor("partial", [T, EMB], f32, kind="ExternalOutput").rearrange(
        "(n p) m -> n p m", p=128
    )

    with tile.TileContext(nc) as tc:
        with (
            tc.tile_pool(name="big", bufs=1) as big,
            tc.tile_pool(name="pt", bufs=26) as ptp,
            tc.tile_pool(name="onorm", bufs=18) as onp,
            tc.tile_pool(name="ostage", bufs=3) as osp,
            tc.tile_pool(name="small", bufs=6) as smp,
            tc.tile_pool(name="mm", bufs=4, space="PSUM") as pmm,
            tc.tile_pool(name="oext", bufs=2, space="PSUM") as pox,
            tc.tile_pool(name="tr", bufs=2, space="PSUM") as ptr,
        ):
            xT_sb = big.tile([128, NE * T], bf16)
            wq_sb = big.tile([128, NE * GS * HD], bf16)
            wk_sb = big.tile([128, NE * HD], bf16)
            wv_sb = big.tile([128, NE * HD], bf16)
            wp_sb = big.tile([128, GS * EMB], bf16)
            qT_sb = big.tile([128, GS * T], bf16)
            kT_sb = big.tile([128, T], bf16)
            vT_sb = big.tile([128, T], bf16)
            vext_sb = big.tile([128, NT * (HD + 1)], bf16)
            ohT_sb = big.tile([128, GS * T], bf16)
            ident = big.tile([128, 128], bf16)
            mask = big.tile([128, 128], bf16)

            # constants: identity for PE transpose; triangular mask for the
            # first 128-col band of diagonal blocks (keep iff q_local >= k_local)
            make_identity(nc, ident)
            nc.gpsimd.memset(mask, 1.0)
            nc.gpsimd.affine_select(
                out=mask,
                in_=mask,
                compare_op=mybir.AluOpType.is_ge,
                fill=0.0,
                base=0,
                pattern=[[1, 128]],
                channel_multiplier=-1,
            )
            nc.vector.memset(vext_sb, 1.0)

            # input DMAs: contiguous block loads; xT quarters gate the
            # projection chains so they go first on the sync queue
            nc.scalar.dma_start(out=wk_sb, in_=wk_d[:, :])
            nc.scalar.dma_start(out=wv_sb, in_=wv_d[:, :])
            xpieces = [1, 3, 4, 4, 4]
            off = 0
            for npc in xpieces:
                w = npc * T
                nc.sync.dma_start(
                    out=xT_sb[:, off : off + w], in_=xT_d[:, off : off + w]
                )
                off += w
            hw = NE * GS * HD // 2
            for q in range(2):
                nc.scalar.dma_start(
                    out=wq_sb[:, q * hw : (q + 1) * hw],
                    in_=wq_d[:, q * hw : (q + 1) * hw],
                )
            nc.scalar.dma_start(out=wp_sb, in_=wp_d[:, :])

            # kT projection: 4 parallel psum chains (tp), contraction chunk outer
            pss = [pmm.tile([128, 512], f32, tag="mm", name=f"pss{i}") for i in range(4)]
            for c in range(NE):
                for tp in range(4):
                    nc.tensor.matmul(
                        pss[tp],
                        lhsT=wk_sb[:, c * HD : (c + 1) * HD],
                        rhs=xT_sb[:, c * T + tp * 512 : c * T + (tp + 1) * 512],
                        start=(c == 0),
                        stop=(c == NE - 1),
                    )
            for tp in range(4):
                nc.scalar.copy(kT_sb[:, tp * 512 : (tp + 1) * 512], pss[tp])

            # vT projection (same shape as kT), then PE-transpose into v tiles
            pss = [pmm.tile([128, 512], f32, tag="mm", name=f"pss{i}") for i in range(4)]
            for c in range(NE):
                for tp in range(4):
                    nc.tensor.matmul(
                        pss[tp],
                        lhsT=wv_sb[:, c * HD : (c + 1) * HD],
                        rhs=xT_sb[:, c * T + tp * 512 : c * T + (tp + 1) * 512],
                        start=(c == 0),
                        stop=(c == NE - 1),
                    )
            for tp in range(4):
                nc.scalar.copy(vT_sb[:, tp * 512 : (tp + 1) * 512], pss[tp])
            for tt in range(NT):
                tv = ptr.tile([128, 128], bf16, tag="tr")
                nc.tensor.transpose(tv, vT_sb[:, tt * 128 : (tt + 1) * 128], ident)
                nc.vector.tensor_copy(
                    vext_sb[:, tt * (HD + 1) : tt * (HD + 1) + HD], tv
                )

            # qT projection per head: 4 parallel psum chains each
            for s in range(GS):
                pss = [pmm.tile([128, 512], f32, tag="mm", name=f"pss{i}") for i in range(4)]
                for c in range(NE):
                    for tp in range(4):
                        nc.tensor.matmul(
                            pss[tp],
                            lhsT=wq_sb[
                                :, c * GS * HD + s * HD : c * GS * HD + (s + 1) * HD
                            ],
                            rhs=xT_sb[:, c * T + tp * 512 : c * T + (tp + 1) * 512],
                            start=(c == 0),
                            stop=(c == NE - 1),
                        )
                for tp in range(4):
                    nc.scalar.copy(
                        qT_sb[:, s * T + tp * 512 : s * T + (tp + 1) * 512], pss[tp]
                    )

            # attention + output projection, software-pipelined: scores for
            # iteration i+1 are emitted before AV of iteration i so the PE
            # stream never waits for ACT's exp backlog at AV chain heads
            deferred = []

            def emit_scores(qp, s):
                pts = []
                for j in range(4 * qp + 4):
                    o = j - 4 * qp  # diagonal offset; <0 for full blocks
                    trim = 128 * o if o > 0 else 0
                    w = 512 - trim
                    q0 = s * T + qp * 512 + trim
                    ps = pmm.tile([128, w], f32, tag="mm", name="ps")
                    nc.tensor.matmul(
                        ps,
                        lhsT=kT_sb[:, j * 128 : (j + 1) * 128],
                        rhs=qT_sb[:, q0 : q0 + w],
                        start=True,
                        stop=True,
                    )
                    pt = ptp.tile([128, w], bf16, tag="pt", name="pt")
                    nc.scalar.activation(
                        pt, ps, mybir.ActivationFunctionType.Exp, scale=SCALE
                    )
                    if o >= 0:
                        # only the first 128-col band straddles the diagonal
                        nc.vector.tensor_mul(pt[:, 0:128], pt[:, 0:128], mask)
                    pts.append((pt, trim))
                return pts

            def emit_av(qp, s, pts):
                for u in range(4):
                    jmax = 4 * qp + u
                    oe = pox.tile([128, HD + 1], f32, tag="oext", name="oe")
                    for j in range(jmax + 1):
                        pt, trim = pts[j]
                        c0 = u * 128 - trim
                        nc.tensor.matmul(
                            oe,
                            lhsT=pt[:, c0 : c0 + 128],
                            rhs=vext_sb[:, j * (HD + 1) : (j + 1) * (HD + 1)],
                            start=(j == 0),
                            stop=(j == jmax),
                        )
                    rc = smp.tile([128, 1], f32, tag="rc", name="rc")
                    nc.vector.reciprocal(rc, oe[:, HD : HD + 1])
                    on = onp.tile([128, 128], bf16, tag="on", name="on")
                    nc.vector.tensor_scalar_mul(on, oe[:, 0:HD], rc)
                    deferred.append((on, s, qp * 512 + u * 128))

            iters = [(qp, s) for qp in range(NQP) for s in range(GS)]
            pending = None  # (qp, s, pts) with scores emitted, AV not yet
            for qp in range(NQP):
                for s in range(GS):
                    pts = emit_scores(qp, s)
                    if pending is not None:
                        emit_av(*pending)
                    pending = (qp, s, pts)
            emit_av(*pending)

            # epilogue per q-pass: transposes (normalizes long done, no PE
            # stall), then the output projection and staged DMA-out
            for qp in range(NQP):
                for on, s, tq in deferred:
                    if tq // 512 != qp:
                        continue
                    tps = ptr.tile([128, 128], bf16, tag="tr", name="tps")
                    nc.tensor.transpose(tps, on, ident)
                    nc.scalar.copy(ohT_sb[:, s * T + tq : s * T + tq + 128], tps)
                for u in range(4):
                    tt = qp * 4 + u
                    for half in range(2):
                        ot = osp.tile([128, EMB // 2], f32, tag="ostage", name="ot")
                        for jph in range(2):
                            jp = half * 2 + jph
                            ps = pmm.tile([128, 512], f32, tag="mm", name="ps")
                            for s in range(GS):
                                nc.tensor.matmul(
                                    ps,
                                    lhsT=ohT_sb[:, s * T + tt * 128 : s * T + (tt + 1) * 128],
                                    rhs=wp_sb[:, s * EMB + jp * 512 : s * EMB + (jp + 1) * 512],
                                    start=(s == 0),
                                    stop=(s == GS - 1),
                                )
                            if jp % 2 == 0:
                                nc.vector.tensor_copy(
                                    ot[:, jph * 512 : (jph + 1) * 512], ps
                                )
                            else:
                                nc.scalar.copy(
                                    ot[:, jph * 512 : (jph + 1) * 512], ps
                                )
                        eng = nc.sync if tt % 2 == 0 else nc.scalar
                        eng.dma_start(
                            out=out_d[tt, :, half * 1024 : (half + 1) * 1024], in_=ot
                        )

    nc.finalize()
    return nc


def _get_program():
    global _PROGRAM
    if _PROGRAM is None:
        _PROGRAM = _build_program()
    return _PROGRAM


def _pack(a, nchunk):
    """[nchunk*128, F] -> [128, nchunk*F] so it lands in SBUF layout with one
    contiguous DMA: out[p, c*F + f] = a[c*128 + p, f]."""
    n, f = a.shape
    assert n == nchunk * 128
    return np.ascontiguousarray(
        a.reshape(nchunk, 128, f).transpose(1, 0, 2).reshape(128, nchunk * f)
    )


def _make_in_maps(x, Wq, Wk, Wv, Wp):
    in_maps = []
    xTs = [_pack(np.asarray(x[b]).T.astype(_BF16), NE) for b in range(2)]
    for c in range(8):
        b, g = c // 4, c % 4
        sl = slice(g * GS * HD, (g + 1) * GS * HD)
        kv = slice(g * GS * HD, g * GS * HD + HD)
        in_maps.append(
            {
                "xTp": xTs[b],
                "wqp": _pack(np.asarray(Wq[sl, :]).T.astype(_BF16), NE),
                "wkp": _pack(np.asarray(Wk[kv, :]).T.astype(_BF16), NE),
                "wvp": _pack(np.asarray(Wv[kv, :]).T.astype(_BF16), NE),
                "wpp": _pack(np.asarray(Wp[:, sl]).T.astype(_BF16), GS),
            }
        )
    return in_maps


def run(x, Wq, Wk, Wv, Wp, bp, trace=False, **trace_kwargs):
    from concourse.bass_utils import run_bass_kernel_spmd

    nc = _get_program()
    in_maps = _make_in_maps(x, Wq, Wk, Wv, Wp)
    res = run_bass_kernel_spmd(
        nc, in_maps, core_ids=list(range(8)), trace=trace, **trace_kwargs
    )
    bp = np.asarray(bp, dtype=np.float32)
    y = np.empty((2, T, EMB), dtype=np.float32)
    for b in range(2):
        acc = res.results[4 * b]["partial"].copy()
        for g in range(1, 4):
            acc += res.results[4 * b + g]["partial"]
        y[b] = acc + bp
    return y, res


def kernel(x, Wq, Wk, Wv, Wp, bp):
    y, _ = run(x, Wq, Wk, Wv, Wp, bp, trace=False)
    return y


# revision 9
# speedup vs baseline: 1.0132x; 1.0132x over previous
"""GQA kernel for Trainium2, 8 NeuronCores.

Sharding: core c = (b, g) with b = c // 4 (batch), g = c % 4 (KV group).
Each core computes, for its batch b and group g (4 query heads, 1 KV head):
  qT[d, t] for the 4 heads, kT[d, t], v[t, d] projections (contraction over EMB,
  inputs pre-packed on host so EMB lands on SBUF partitions),
  causal flash-style attention in [k-part, q-free] score layout,
  and the partial output projection  partial_g = (attn out) @ Wp[:, g cols].T.
Host gathers: y[b] = sum_g partial[b, g] + bp.

All matmuls run in bf16 (fp32 PSUM accumulation); host pre-casts inputs.
Inputs are host-packed to the exact SBUF layout [128, free] so each tensor
loads with one contiguous DMA (dma_start issue cost dominates chunked loads).
Causal structure: scores for the diagonal 128-row k-blocks are trimmed to the
q-columns that can attend; only the first 128-col band of each diagonal block
needs an elementwise triangular mask.
"""

import numpy as np
import ml_dtypes

T = 2048
EMB = 2048
HD = 128
GS = 4          # query heads per core (per KV group)
NE = EMB // 128 # 16 contraction chunks
NT = T // 128   # 16 row tiles
NQP = T // 512  # 4 q passes of 512
SCALE = float(HD) ** -0.5

_BF16 = ml_dtypes.bfloat16
_PROGRAM = None


def _build_program():
    import concourse.bass as bass
    import concourse.tile as tile
    from concourse import bacc, mybir
    from concourse.masks import make_identity

    f32 = mybir.dt.float32
    bf16 = mybir.dt.bfloat16

    nc = bacc.Bacc("TRN2", target_bir_lowering=False, debug=False)

    # all inputs host-packed to [128 partitions, free] SBUF layout
    xT_d = nc.dram_tensor("xTp", [128, NE * T], bf16, kind="ExternalInput")
    wq_d = nc.dram_tensor("wqp", [128, NE * GS * HD], bf16, kind="ExternalInput")
    wk_d = nc.dram_tensor("wkp", [128, NE * HD], bf16, kind="ExternalInput")
    wv_d = nc.dram_tensor("wvp", [128, NE * HD], bf16, kind="ExternalInput")
    wp_d = nc.dram_tensor("wpp", [128, GS * EMB], bf16, kind="ExternalInput")
    out_d = nc.dram_tensor("partial", [T, EMB], f32, kind="ExternalOutput").rearrange(
        "(n p) m -> n p m", p=128
    )

    with tile.TileContext(nc) as tc:
        with (
            tc.tile_pool(name="big", bufs=1) as big,
            tc.tile_pool(name="pt", bufs=26) as ptp,
            tc.tile_pool(name="onorm", bufs=18) as onp,
            tc.tile_pool(name="ostage", bufs=3) as osp,
            tc.tile_pool(name="small", bufs=6) as smp,
            tc.tile_pool(name="mm", bufs=4, space="PSUM") as pmm,
            tc.tile_pool(name="oext", bufs=2, space="PSUM") as pox,
            tc.tile_pool(name="tr", bufs=2, space="PSUM") as ptr,
        ):
            xT_sb = big.tile([128, NE * T], bf16)
            wq_sb = big.tile([128, NE * GS * HD], bf16)
            wk_sb = big.tile([128, NE * HD], bf16)
            wv_sb = big.tile([128, NE * HD], bf16)
            wp_sb = big.tile([128, GS * EMB], bf16)
            qT_sb = big.tile([128, GS * T], bf16)
            kT_sb = big.tile([128, T], bf16)
            vT_sb = big.tile([128, T], bf16)
            vext_sb = big.tile([128, NT * (HD + 1)], bf16)
            ohT_sb = big.tile([128, GS * T], bf16)
            ident = big.tile([128, 128], bf16)
            mask = big.tile([128, 128], bf16)

            # constants: identity for PE transpose; triangular mask for the
            # first 128-col band of diagonal blocks (keep iff q_local >= k_local)
            make_identity(nc, ident)
            nc.gpsimd.memset(mask, 1.0)
            nc.gpsimd.affine_select(
                out=mask,
                in_=mask,
                compare_op=mybir.AluOpType.is_ge,
                fill=0.0,
                base=0,
                pattern=[[1, 128]],
                channel_multiplier=-1,
            )
            nc.vector.memset(vext_sb, 1.0)

            # input DMAs: contiguous block loads; xT quarters gate the
            # projection chains so they go first on the sync queue
            nc.scalar.dma_start(out=wk_sb, in_=wk_d[:, :])
            nc.scalar.dma_start(out=wv_sb, in_=wv_d[:, :])
            xpieces = [1, 3, 4, 4, 4]
            off = 0
            for npc in xpieces:
                w = npc * T
                nc.sync.dma_start(
                    out=xT_sb[:, off : off + w], in_=xT_d[:, off : off + w]
                )
                off += w
            hw = NE * GS * HD // 2
            for q in range(2):
                nc.scalar.dma_start(
                    out=wq_sb[:, q * hw : (q + 1) * hw],
                    in_=wq_d[:, q * hw : (q + 1) * hw],
                )
            nc.scalar.dma_start(out=wp_sb, in_=wp_d[:, :])

            # kT projection: 4 parallel psum chains (tp), contraction chunk outer
            pss = [pmm.tile([128, 512], f32, tag="mm", name=f"pss{i}") for i in range(4)]
            for c in range(NE):
                for tp in range(4):
                    nc.tensor.matmul(
                        pss[tp],
                        lhsT=wk_sb[:, c * HD : (c + 1) * HD],
                        rhs=xT_sb[:, c * T + tp * 512 : c * T + (tp + 1) * 512],
                        start=(c == 0),
                        stop=(c == NE - 1),
                    )
            for tp in range(4):
                nc.scalar.copy(kT_sb[:, tp * 512 : (tp + 1) * 512], pss[tp])

            # vT projection (same shape as kT), then PE-transpose into v tiles
            pss = [pmm.tile([128, 512], f32, tag="mm", name=f"pss{i}") for i in range(4)]
            for c in range(NE):
                for tp in range(4):
                    nc.tensor.matmul(
                        pss[tp],
                        lhsT=wv_sb[:, c * HD : (c + 1) * HD],
                        rhs=xT_sb[:, c * T + tp * 512 : c * T + (tp + 1) * 512],
                        start=(c == 0),
                        stop=(c == NE - 1),
                    )
            for tp in range(4):
                nc.scalar.copy(vT_sb[:, tp * 512 : (tp + 1) * 512], pss[tp])
            for tt in range(NT):
                tv = ptr.tile([128, 128], bf16, tag="tr")
                nc.tensor.transpose(tv, vT_sb[:, tt * 128 : (tt + 1) * 128], ident)
                nc.vector.tensor_copy(
                    vext_sb[:, tt * (HD + 1) : tt * (HD + 1) + HD], tv
                )

            # qT projection per head: 4 parallel psum chains each
            for s in range(GS):
                pss = [pmm.tile([128, 512], f32, tag="mm", name=f"pss{i}") for i in range(4)]
                for c in range(NE):
                    for tp in range(4):
                        nc.tensor.matmul(
                            pss[tp],
                            lhsT=wq_sb[
                                :, c * GS * HD + s * HD : c * GS * HD + (s + 1) * HD
                            ],
                            rhs=xT_sb[:, c * T + tp * 512 : c * T + (tp + 1) * 512],
                            start=(c == 0),
                            stop=(c == NE - 1),
                        )
                for tp in range(4):
                    nc.scalar.copy(
                        qT_sb[:, s * T + tp * 512 : s * T + (tp + 1) * 512], pss[tp]
                    )

            # attention + output projection, software-pipelined: scores for
            # iteration i+1 are emitted before AV of iteration i so the PE
            # stream never waits for ACT's exp backlog at AV chain heads
            deferred = []

            def emit_scores(qp, s):
                pts = []
                for j in range(4 * qp + 4):
                    o = j - 4 * qp  # diagonal offset; <0 for full blocks
                    trim = 128 * o if o > 0 else 0
                    w = 512 - trim
                    q0 = s * T + qp * 512 + trim
                    ps = pmm.tile([128, w], f32, tag="mm", name="ps")
                    nc.tensor.matmul(
                        ps,
                        lhsT=kT_sb[:, j * 128 : (j + 1) * 128],
                        rhs=qT_sb[:, q0 : q0 + w],
                        start=True,
                        stop=True,
                    )
                    pt = ptp.tile([128, w], bf16, tag="pt", name="pt")
                    nc.scalar.activation(
                        pt, ps, mybir.ActivationFunctionType.Exp, scale=SCALE
                    )
                    if o >= 0:
                        # only the first 128-col band straddles the diagonal
                        nc.vector.tensor_mul(pt[:, 0:128], pt[:, 0:128], mask)
                    pts.append((pt, trim))
                return pts

            def emit_av(qp, s, pts):
                for u in range(4):
                    jmax = 4 * qp + u
                    oe = pox.tile([128, HD + 1], f32, tag="oext", name="oe")
                    for j in range(jmax + 1):
                        pt, trim = pts[j]
                        c0 = u * 128 - trim
                        nc.tensor.matmul(
                            oe,
                            lhsT=pt[:, c0 : c0 + 128],
                            rhs=vext_sb[:, j * (HD + 1) : (j + 1) * (HD + 1)],
                            start=(j == 0),
                            stop=(j == jmax),
                        )
                    rc = smp.tile([128, 1], f32, tag="rc", name="rc")
                    nc.vector.reciprocal(rc, oe[:, HD : HD + 1])
                    on = onp.tile([128, 128], bf16, tag="on", name="on")
                    nc.vector.tensor_scalar_mul(on, oe[:, 0:HD], rc)
                    deferred.append((on, s, qp * 512 + u * 128))

            def emit_epilogue(qp):
                # transposes (normalizes long done, no PE stall), then the
                # output projection for this q-pass's 4 row tiles
                mine = [d for d in deferred if d[2] // 512 == qp]
                deferred[:] = [d for d in deferred if d[2] // 512 != qp]
                for on, s, tq in mine:
                    tps = ptr.tile([128, 128], bf16, tag="tr", name="tps")
                    nc.tensor.transpose(tps, on, ident)
                    nc.scalar.copy(ohT_sb[:, s * T + tq : s * T + tq + 128], tps)
                for u in range(4):
                    tt = qp * 4 + u
                    for half in range(2):
                        ot = osp.tile([128, EMB // 2], f32, tag="ostage", name="ot")
                        for jph in range(2):
                            jp = half * 2 + jph
                            ps = pmm.tile([128, 512], f32, tag="mm", name="ps")
                            for s in range(GS):
                                nc.tensor.matmul(
                                    ps,
                                    lhsT=ohT_sb[:, s * T + tt * 128 : s * T + (tt + 1) * 128],
                                    rhs=wp_sb[:, s * EMB + jp * 512 : s * EMB + (jp + 1) * 512],
                                    start=(s == 0),
                                    stop=(s == GS - 1),
                                )
                            if jp % 2 == 0:
                                nc.vector.tensor_copy(
                                    ot[:, jph * 512 : (jph + 1) * 512], ps
                                )
                            else:
                                nc.scalar.copy(
                                    ot[:, jph * 512 : (jph + 1) * 512], ps
                                )
                        eng = nc.sync if tt % 2 == 0 else nc.scalar
                        eng.dma_start(
                            out=out_d[tt, :, half * 1024 : (half + 1) * 1024], in_=ot
                        )

            pending = None  # (qp, s, pts) with scores emitted, AV not yet
            for qp in range(NQP):
                for s in range(GS):
                    pts = emit_scores(qp, s)
                    if pending is not None:
                        emit_av(*pending)
                        if pending[1] == GS - 1:
                            emit_epilogue(pending[0])
                    pending = (qp, s, pts)
            emit_av(*pending)
            emit_epilogue(pending[0])

    nc.finalize()
    return nc


def _get_program():
    global _PROGRAM
    if _PROGRAM is None:
        _PROGRAM = _build_program()
    return _PROGRAM


def _pack(a, nchunk):
    """[nchunk*128, F] -> [128, nchunk*F] so it lands in SBUF layout with one
    contiguous DMA: out[p, c*F + f] = a[c*128 + p, f]."""
    n, f = a.shape
    assert n == nchunk * 128
    return np.ascontiguousarray(
        a.reshape(nchunk, 128, f).transpose(1, 0, 2).reshape(128, nchunk * f)
    )


def _make_in_maps(x, Wq, Wk, Wv, Wp):
    in_maps = []
    xTs = [_pack(np.asarray(x[b]).T.astype(_BF16), NE) for b in range(2)]
    for c in range(8):
        b, g = c // 4, c % 4
        sl = slice(g * GS * HD, (g + 1) * GS * HD)
        kv = slice(g * GS * HD, g * GS * HD + HD)
        in_maps.append(
            {
                "xTp": xTs[b],
                "wqp": _pack(np.asarray(Wq[sl, :]).T.astype(_BF16), NE),
                "wkp": _pack(np.asarray(Wk[kv, :]).T.astype(_BF16), NE),
                "wvp": _pack(np.asarray(Wv[kv, :]).T.astype(_BF16), NE),
                "wpp": _pack(np.asarray(Wp[:, sl]).T.astype(_BF16), GS),
            }
        )
    return in_maps


def run(x, Wq, Wk, Wv, Wp, bp, trace=False, **trace_kwargs):
    from concourse.bass_utils import run_bass_kernel_spmd

    nc = _get_program()
    in_maps = _make_in_maps(x, Wq, Wk, Wv, Wp)
    res = run_bass_kernel_spmd(
        nc, in_maps, core_ids=list(range(8)), trace=trace, **trace_kwargs
    )
    bp = np.asarray(bp, dtype=np.float32)
    y = np.empty((2, T, EMB), dtype=np.float32)
    for b in range(2):
        acc = res.results[4 * b]["partial"].copy()
        for g in range(1, 4):
            acc += res.results[4 * b + g]["partial"]
        y[b] = acc + bp
    return y, res


def kernel(x, Wq, Wk, Wv, Wp, bp):
    y, _ = run(x, Wq, Wk, Wv, Wp, bp, trace=False)
    return y


# revision 11
# speedup vs baseline: 1.0840x; 1.0700x over previous
"""GQA kernel for Trainium2, 8 NeuronCores.

Sharding: core c = (b, g) with b = c // 4 (batch), g = c % 4 (KV group).
Each core computes, for its batch b and group g (4 query heads, 1 KV head):
  qT[d, t] for the 4 heads, kT[d, t], v[t, d] projections (contraction over EMB,
  inputs pre-packed on host so EMB lands on SBUF partitions),
  causal flash-style attention in [k-part, q-free] score layout,
  and the partial output projection  partial_g = (attn out) @ Wp[:, g cols].T.
Host gathers: y[b] = sum_g partial[b, g] + bp.

All matmuls run in bf16 (fp32 PSUM accumulation); host pre-casts inputs.
Inputs are host-packed to the exact SBUF layout [128, free] so each tensor
loads with one contiguous DMA (dma_start issue cost dominates chunked loads).
Causal structure: scores for the diagonal 128-row k-blocks are trimmed to the
q-columns that can attend; only the first 128-col band of each diagonal block
needs an elementwise triangular mask.
"""

import numpy as np
import ml_dtypes

T = 2048
EMB = 2048
HD = 128
GS = 4          # query heads per core (per KV group)
NE = EMB // 128 # 16 contraction chunks
NT = T // 128   # 16 row tiles
NQP = T // 512  # 4 q passes of 512
SCALE = float(HD) ** -0.5

_BF16 = ml_dtypes.bfloat16
_PROGRAM = None


def _build_program():
    import concourse.bass as bass
    import concourse.tile as tile
    from concourse import bacc, mybir
    from concourse.masks import make_identity

    f32 = mybir.dt.float32
    bf16 = mybir.dt.bfloat16

    nc = bacc.Bacc("TRN2", target_bir_lowering=False, debug=False)

    # all inputs host-packed to [128 partitions, free] SBUF layout
    xT_d = nc.dram_tensor("xTp", [128, NE * T], bf16, kind="ExternalInput")
    wq_d = nc.dram_tensor("wqp", [128, NE * GS * HD], bf16, kind="ExternalInput")
    wk_d = nc.dram_tensor("wkp", [128, NE * HD], bf16, kind="ExternalInput")
    wv_d = nc.dram_tensor("wvp", [128, NE * HD], bf16, kind="ExternalInput")
    wp_d = nc.dram_tensor("wpp", [128, GS * EMB], bf16, kind="ExternalInput")
    out_d = nc.dram_tensor("partial", [T, EMB], f32, kind="ExternalOutput").rearrange(
        "(n p) m -> n p m", p=128
    )

    with tile.TileContext(nc) as tc:
        with (
            tc.tile_pool(name="big", bufs=1) as big,
            tc.tile_pool(name="pt", bufs=26) as ptp,
            tc.tile_pool(name="onorm", bufs=18) as onp,
            tc.tile_pool(name="ostage", bufs=3) as osp,
            tc.tile_pool(name="small", bufs=6) as smp,
            tc.tile_pool(name="mm", bufs=4, space="PSUM") as pmm,
            tc.tile_pool(name="oext", bufs=2, space="PSUM") as pox,
            tc.tile_pool(name="tr", bufs=2, space="PSUM") as ptr,
        ):
            xT_sb = big.tile([128, NE * T], bf16)
            wq_sb = big.tile([128, NE * GS * HD], bf16)
            wk_sb = big.tile([128, NE * HD], bf16)
            wv_sb = big.tile([128, NE * HD], bf16)
            wp_sb = big.tile([128, GS * EMB], bf16)
            qT_sb = big.tile([128, GS * T], bf16)
            kT_sb = big.tile([128, T], bf16)
            vT_sb = big.tile([128, T], bf16)
            vext_sb = big.tile([128, NT * (HD + 1)], bf16)
            ohT_sb = big.tile([128, GS * T], bf16)
            ident = big.tile([128, 128], bf16)
            mask = big.tile([128, 128], bf16)

            # constants: identity for PE transpose; triangular mask for the
            # first 128-col band of diagonal blocks (keep iff q_local >= k_local)
            make_identity(nc, ident)
            nc.gpsimd.memset(mask, 1.0)
            nc.gpsimd.affine_select(
                out=mask,
                in_=mask,
                compare_op=mybir.AluOpType.is_ge,
                fill=0.0,
                base=0,
                pattern=[[1, 128]],
                channel_multiplier=-1,
            )
            nc.vector.memset(vext_sb, 1.0)

            # input DMAs: contiguous block loads; xT quarters gate the
            # projection chains so they go first on the sync queue
            nc.scalar.dma_start(out=wk_sb, in_=wk_d[:, :])
            nc.scalar.dma_start(out=wv_sb, in_=wv_d[:, :])
            for c in range(NE):
                nc.sync.dma_start(
                    out=xT_sb[:, c * T : (c + 1) * T], in_=xT_d[:, c * T : (c + 1) * T]
                )
            hw = NE * GS * HD // 2
            for q in range(2):
                nc.scalar.dma_start(
                    out=wq_sb[:, q * hw : (q + 1) * hw],
                    in_=wq_d[:, q * hw : (q + 1) * hw],
                )
            nc.scalar.dma_start(out=wp_sb, in_=wp_d[:, :])

            # kT + vT projections interleaved, chunk-outer so PE consumes each
            # xT chunk as it arrives. kT uses the 4 "mm" psum slots; vT borrows
            # the attention pools' slots (oext x2 + tr x2) so both run during
            # the DMA-arrival window.
            kss = [pmm.tile([128, 512], f32, tag="mm", name=f"kss{i}") for i in range(4)]
            vss = [
        pox.tile([128, 512], f32, tag="oext", name="vss0"),
        pox.tile([128, 512], f32, tag="oext", name="vss1"),
        ptr.tile([128, 512], f32, tag="tr", name="vss2"),
        ptr.tile([128, 512], f32, tag="tr", name="vss3"),
            ]
            for c in range(NE):
                for tp in range(4):
                    nc.tensor.matmul(
                        kss[tp],
                        lhsT=wk_sb[:, c * HD : (c + 1) * HD],
                        rhs=xT_sb[:, c * T + tp * 512 : c * T + (tp + 1) * 512],
                        start=(c == 0),
                        stop=(c == NE - 1),
                    )
                for tp in range(4):
                    nc.tensor.matmul(
                        vss[tp],
                        lhsT=wv_sb[:, c * HD : (c + 1) * HD],
                        rhs=xT_sb[:, c * T + tp * 512 : c * T + (tp + 1) * 512],
                        start=(c == 0),
                        stop=(c == NE - 1),
                    )
            for tp in range(4):
                nc.scalar.copy(kT_sb[:, tp * 512 : (tp + 1) * 512], kss[tp])
            for tp in range(4):
                nc.scalar.copy(vT_sb[:, tp * 512 : (tp + 1) * 512], vss[tp])
            for tt in range(NT):
                tv = ptr.tile([128, 128], bf16, tag="tr")
                nc.tensor.transpose(tv, vT_sb[:, tt * 128 : (tt + 1) * 128], ident)
                nc.vector.tensor_copy(
                    vext_sb[:, tt * (HD + 1) : tt * (HD + 1) + HD], tv
                )

            # qT projection per head: 4 parallel psum chains each
            for s in range(GS):
                pss = [pmm.tile([128, 512], f32, tag="mm", name=f"pss{i}") for i in range(4)]
                for c in range(NE):
                    for tp in range(4):
                        nc.tensor.matmul(
                            pss[tp],
                            lhsT=wq_sb[
                                :, c * GS * HD + s * HD : c * GS * HD + (s + 1) * HD
                            ],
                            rhs=xT_sb[:, c * T + tp * 512 : c * T + (tp + 1) * 512],
                            start=(c == 0),
                            stop=(c == NE - 1),
                        )
                for tp in range(4):
                    nc.scalar.copy(
                        qT_sb[:, s * T + tp * 512 : s * T + (tp + 1) * 512], pss[tp]
                    )

            # attention + output projection, software-pipelined: scores for
            # iteration i+1 are emitted before AV of iteration i so the PE
            # stream never waits for ACT's exp backlog at AV chain heads
            deferred = []

            def emit_scores(qp, s):
                pts = []
                for j in range(4 * qp + 4):
                    o = j - 4 * qp  # diagonal offset; <0 for full blocks
                    trim = 128 * o if o > 0 else 0
                    w = 512 - trim
                    q0 = s * T + qp * 512 + trim
                    ps = pmm.tile([128, w], f32, tag="mm", name="ps")
                    nc.tensor.matmul(
                        ps,
                        lhsT=kT_sb[:, j * 128 : (j + 1) * 128],
                        rhs=qT_sb[:, q0 : q0 + w],
                        start=True,
                        stop=True,
                    )
                    pt = ptp.tile([128, w], bf16, tag="pt", name="pt")
                    nc.scalar.activation(
                        pt, ps, mybir.ActivationFunctionType.Exp, scale=SCALE
                    )
                    if o >= 0:
                        # only the first 128-col band straddles the diagonal
                        nc.vector.tensor_mul(pt[:, 0:128], pt[:, 0:128], mask)
                    pts.append((pt, trim))
                return pts

            def emit_av(qp, s, pts):
                norms = []
                for u in range(4):
                    jmax = 4 * qp + u
                    oe = pox.tile([128, HD + 1], f32, tag="oext", name="oe")
                    for j in range(jmax + 1):
                        pt, trim = pts[j]
                        c0 = u * 128 - trim
                        nc.tensor.matmul(
                            oe,
                            lhsT=pt[:, c0 : c0 + 128],
                            rhs=vext_sb[:, j * (HD + 1) : (j + 1) * (HD + 1)],
                            start=(j == 0),
                            stop=(j == jmax),
                        )
                    rc = smp.tile([128, 1], f32, tag="rc", name="rc")
                    nc.vector.reciprocal(rc, oe[:, HD : HD + 1])
                    on = onp.tile([128, 128], bf16, tag="on", name="on")
                    nc.vector.tensor_scalar_mul(on, oe[:, 0:HD], rc)
                    norms.append((on, s, qp * 512 + u * 128))
                return norms

            def emit_transposes(norms):
                for on, s, tq in norms:
                    tps = ptr.tile([128, 128], bf16, tag="tr", name="tps")
                    nc.tensor.transpose(tps, on, ident)
                    nc.scalar.copy(ohT_sb[:, s * T + tq : s * T + tq + 128], tps)

            def emit_epilogue(qp):
                # output projection for this q-pass's 4 row tiles
                for u in range(4):
                    tt = qp * 4 + u
                    for half in range(2):
                        ot = osp.tile([128, EMB // 2], f32, tag="ostage", name="ot")
                        for jph in range(2):
                            jp = half * 2 + jph
                            ps = pmm.tile([128, 512], f32, tag="mm", name="ps")
                            for s in range(GS):
                                nc.tensor.matmul(
                                    ps,
                                    lhsT=ohT_sb[:, s * T + tt * 128 : s * T + (tt + 1) * 128],
                                    rhs=wp_sb[:, s * EMB + jp * 512 : s * EMB + (jp + 1) * 512],
                                    start=(s == 0),
                                    stop=(s == GS - 1),
                                )
                            if jp % 2 == 0:
                                nc.vector.tensor_copy(
                                    ot[:, jph * 512 : (jph + 1) * 512], ps
                                )
                            else:
                                nc.scalar.copy(
                                    ot[:, jph * 512 : (jph + 1) * 512], ps
                                )
                        eng = nc.sync if tt % 2 == 0 else nc.scalar
                        eng.dma_start(
                            out=out_d[tt, :, half * 1024 : (half + 1) * 1024], in_=ot
                        )

            pending = None  # (qp, s, pts) with scores emitted, AV not yet
            last_norms = []

            def advance(pending):
                # AV for the pending iteration, then the (lag-1) transposes of
                # the previous one; at a q-pass boundary flush and project
                nonlocal last_norms
                qp, s, pts = pending
                norms = emit_av(qp, s, pts)
                emit_transposes(last_norms)
                last_norms = norms
                if s == GS - 1:
                    emit_transposes(last_norms)
                    last_norms = []
                    emit_epilogue(qp)

            for qp in range(NQP):
                for s in range(GS):
                    pts = emit_scores(qp, s)
                    if pending is not None:
                        advance(pending)
                    pending = (qp, s, pts)
            advance(pending)

    nc.finalize()
    return nc


def _get_program():
    global _PROGRAM
    if _PROGRAM is None:
        _PROGRAM = _build_program()
    return _PROGRAM


def _pack(a, nchunk):
    """[nchunk*128, F] -> [128, nchunk*F] so it lands in SBUF layout with one
    contiguous DMA: out[p, c*F + f] = a[c*128 + p, f]."""
    n, f = a.shape
    assert n == nchunk * 128
    return np.ascontiguousarray(
        a.reshape(nchunk, 128, f).transpose(1, 0, 2).reshape(128, nchunk * f)
    )


def _make_in_maps(x, Wq, Wk, Wv, Wp):
    in_maps = []
    xTs = [_pack(np.asarray(x[b]).T.astype(_BF16), NE) for b in range(2)]
    for c in range(8):
        b, g = c // 4, c % 4
        sl = slice(g * GS * HD, (g + 1) * GS * HD)
        kv = slice(g * GS * HD, g * GS * HD + HD)
        in_maps.append(
            {
                "xTp": xTs[b],
                "wqp": _pack(np.asarray(Wq[sl, :]).T.astype(_BF16), NE),
                "wkp": _pack(np.asarray(Wk[kv, :]).T.astype(_BF16), NE),
                "wvp": _pack(np.asarray(Wv[kv, :]).T.astype(_BF16), NE),
                "wpp": _pack(np.asarray(Wp[:, sl]).T.astype(_BF16), GS),
            }
        )
    return in_maps


def run(x, Wq, Wk, Wv, Wp, bp, trace=False, **trace_kwargs):
    from concourse.bass_utils import run_bass_kernel_spmd

    nc = _get_program()
    in_maps = _make_in_maps(x, Wq, Wk, Wv, Wp)
    res = run_bass_kernel_spmd(
        nc, in_maps, core_ids=list(range(8)), trace=trace, **trace_kwargs
    )
    bp = np.asarray(bp, dtype=np.float32)
    y = np.empty((2, T, EMB), dtype=np.float32)
    for b in range(2):
        acc = res.results[4 * b]["partial"].copy()
        for g in range(1, 4):
            acc += res.results[4 * b + g]["partial"]
        y[b] = acc + bp
    return y, res


def kernel(x, Wq, Wk, Wv, Wp, bp):
    y, _ = run(x, Wq, Wk, Wv, Wp, bp, trace=False)
    return y


# revision 13
# speedup vs baseline: 1.1461x; 1.0573x over previous
"""GQA kernel for Trainium2, 8 NeuronCores.

Sharding: core c = (b, g) with b = c // 4 (batch), g = c % 4 (KV group).
Each core computes, for its batch b and group g (4 query heads, 1 KV head):
  qT[d, t] for the 4 heads, kT[d, t], v[t, d] projections (contraction over EMB,
  inputs pre-packed on host so EMB lands on SBUF partitions),
  causal flash-style attention in [k-part, q-free] score layout,
  and the partial output projection  partial_g = (attn out) @ Wp[:, g cols].T.
Host gathers: y[b] = sum_g partial[b, g] + bp.

All matmuls run in bf16 (fp32 PSUM accumulation); host pre-casts inputs.
Inputs are host-packed to the exact SBUF layout [128, free] so each tensor
loads with one contiguous DMA (dma_start issue cost dominates chunked loads).
Causal structure: scores for the diagonal 128-row k-blocks are trimmed to the
q-columns that can attend; only the first 128-col band of each diagonal block
needs an elementwise triangular mask.
"""

import numpy as np
import ml_dtypes

T = 2048
EMB = 2048
HD = 128
GS = 4          # query heads per core (per KV group)
NE = EMB // 128 # 16 contraction chunks
NT = T // 128   # 16 row tiles
NQP = T // 512  # 4 q passes of 512
SCALE = float(HD) ** -0.5

_BF16 = ml_dtypes.bfloat16
_PROGRAM = None


def _build_program():
    import concourse.bass as bass
    import concourse.tile as tile
    from concourse import bacc, mybir
    from concourse.masks import make_identity

    f32 = mybir.dt.float32
    bf16 = mybir.dt.bfloat16

    nc = bacc.Bacc("TRN2", target_bir_lowering=False, debug=False)

    # all inputs host-packed to [128 partitions, free] SBUF layout
    xT_d = nc.dram_tensor("xTp", [128, NE * T], bf16, kind="ExternalInput")
    wq_d = nc.dram_tensor("wqp", [128, NE * GS * HD], bf16, kind="ExternalInput")
    wk_d = nc.dram_tensor("wkp", [128, NE * HD], bf16, kind="ExternalInput")
    wv_d = nc.dram_tensor("wvp", [128, NE * HD], bf16, kind="ExternalInput")
    wp_d = nc.dram_tensor("wpp", [128, GS * EMB], bf16, kind="ExternalInput")
    out_d = nc.dram_tensor("partial", [T, EMB], f32, kind="ExternalOutput").rearrange(
        "(n p) m -> n p m", p=128
    )

    with tile.TileContext(nc) as tc:
        with (
            tc.tile_pool(name="big", bufs=1) as big,
            tc.tile_pool(name="pt", bufs=26) as ptp,
            tc.tile_pool(name="onorm", bufs=18) as onp,
            tc.tile_pool(name="ostage", bufs=3) as osp,
            tc.tile_pool(name="small", bufs=6) as smp,
            tc.tile_pool(name="mm", bufs=4, space="PSUM") as pmm,
            tc.tile_pool(name="oext", bufs=2, space="PSUM") as pox,
            tc.tile_pool(name="tr", bufs=2, space="PSUM") as ptr,
        ):
            xT_sb = big.tile([128, NE * T], bf16)
            wq_sb = big.tile([128, NE * GS * HD], bf16)
            wk_sb = big.tile([128, NE * HD], bf16)
            wv_sb = big.tile([128, NE * HD], bf16)
            wp_sb = big.tile([128, GS * EMB], bf16)
            qT_sb = big.tile([128, GS * T], bf16)
            kT_sb = big.tile([128, T], bf16)
            vT_sb = big.tile([128, T], bf16)
            vext_sb = big.tile([128, NT * (HD + 1)], bf16)
            ohT_sb = big.tile([128, GS * T], bf16)
            ident = big.tile([128, 128], bf16)
            mask = big.tile([128, 128], bf16)

            # constants: identity for PE transpose; triangular mask for the
            # first 128-col band of diagonal blocks (keep iff q_local >= k_local)
            make_identity(nc, ident)
            nc.gpsimd.memset(mask, 1.0)
            nc.gpsimd.affine_select(
                out=mask,
                in_=mask,
                compare_op=mybir.AluOpType.is_ge,
                fill=0.0,
                base=0,
                pattern=[[1, 128]],
                channel_multiplier=-1,
            )
            nc.vector.memset(vext_sb, 1.0)

            # input DMAs: contiguous block loads; xT quarters gate the
            # projection chains so they go first on the sync queue
            nc.scalar.dma_start(out=wk_sb, in_=wk_d[:, :])
            nc.scalar.dma_start(out=wv_sb, in_=wv_d[:, :])
            for c in range(NE):
                nc.sync.dma_start(
                    out=xT_sb[:, c * T : (c + 1) * T], in_=xT_d[:, c * T : (c + 1) * T]
                )
            hw = NE * GS * HD // 2
            for q in range(2):
                nc.scalar.dma_start(
                    out=wq_sb[:, q * hw : (q + 1) * hw],
                    in_=wq_d[:, q * hw : (q + 1) * hw],
                )
            nc.scalar.dma_start(out=wp_sb, in_=wp_d[:, :])

            # PE clock warmup: ~24 back-to-back matmuls on garbage SBUF data
            # (nothing reads the result) so HAM reaches 2.4 GHz before the
            # first real, DMA-gated matmuls trickle in
            wgarb = big.tile([128, 512], bf16)
            nc.gpsimd.memset(wgarb, 1.0)
            wps = pmm.tile([128, 512], f32, tag="mm", name="wps")
            for _ in range(24):
                nc.tensor.matmul(wps, lhsT=ident, rhs=wgarb, start=True, stop=True)

            # kT + vT projections interleaved, chunk-outer so PE consumes each
            # xT chunk as it arrives. kT uses the 4 "mm" psum slots; vT borrows
            # the attention pools' slots (oext x2 + tr x2) so both run during
            # the DMA-arrival window.
            kss = [pmm.tile([128, 512], f32, tag="mm", name=f"kss{i}") for i in range(4)]
            vss = [
        pox.tile([128, 512], f32, tag="oext", name="vss0"),
        pox.tile([128, 512], f32, tag="oext", name="vss1"),
        ptr.tile([128, 512], f32, tag="tr", name="vss2"),
        ptr.tile([128, 512], f32, tag="tr", name="vss3"),
            ]
            for c in range(NE):
                for tp in range(4):
                    nc.tensor.matmul(
                        kss[tp],
                        lhsT=wk_sb[:, c * HD : (c + 1) * HD],
                        rhs=xT_sb[:, c * T + tp * 512 : c * T + (tp + 1) * 512],
                        start=(c == 0),
                        stop=(c == NE - 1),
                    )
                for tp in range(4):
                    nc.tensor.matmul(
                        vss[tp],
                        lhsT=wv_sb[:, c * HD : (c + 1) * HD],
                        rhs=xT_sb[:, c * T + tp * 512 : c * T + (tp + 1) * 512],
                        start=(c == 0),
                        stop=(c == NE - 1),
                    )
            for tp in range(4):
                nc.scalar.copy(kT_sb[:, tp * 512 : (tp + 1) * 512], kss[tp])
            for tp in range(4):
                nc.scalar.copy(vT_sb[:, tp * 512 : (tp + 1) * 512], vss[tp])
            for tt in range(NT):
                tv = ptr.tile([128, 128], bf16, tag="tr")
                nc.tensor.transpose(tv, vT_sb[:, tt * 128 : (tt + 1) * 128], ident)
                nc.vector.tensor_copy(
                    vext_sb[:, tt * (HD + 1) : tt * (HD + 1) + HD], tv
                )

            # qT projection per head: 4 parallel psum chains each
            for s in range(GS):
                pss = [pmm.tile([128, 512], f32, tag="mm", name=f"pss{i}") for i in range(4)]
                for c in range(NE):
                    for tp in range(4):
                        nc.tensor.matmul(
                            pss[tp],
                            lhsT=wq_sb[
                                :, c * GS * HD + s * HD : c * GS * HD + (s + 1) * HD
                            ],
                            rhs=xT_sb[:, c * T + tp * 512 : c * T + (tp + 1) * 512],
                            start=(c == 0),
                            stop=(c == NE - 1),
                        )
                for tp in range(4):
                    nc.scalar.copy(
                        qT_sb[:, s * T + tp * 512 : s * T + (tp + 1) * 512], pss[tp]
                    )

            # attention + output projection, software-pipelined: scores for
            # iteration i+1 are emitted before AV of iteration i so the PE
            # stream never waits for ACT's exp backlog at AV chain heads
            deferred = []

            def emit_scores(qp, s):
                pts = []
                for j in range(4 * qp + 4):
                    o = j - 4 * qp  # diagonal offset; <0 for full blocks
                    trim = 128 * o if o > 0 else 0
                    w = 512 - trim
                    q0 = s * T + qp * 512 + trim
                    ps = pmm.tile([128, w], f32, tag="mm", name="ps")
                    nc.tensor.matmul(
                        ps,
                        lhsT=kT_sb[:, j * 128 : (j + 1) * 128],
                        rhs=qT_sb[:, q0 : q0 + w],
                        start=True,
                        stop=True,
                    )
                    pt = ptp.tile([128, w], bf16, tag="pt", name="pt")
                    nc.scalar.activation(
                        pt, ps, mybir.ActivationFunctionType.Exp, scale=SCALE
                    )
                    if o >= 0:
                        # only the first 128-col band straddles the diagonal
                        nc.vector.tensor_mul(pt[:, 0:128], pt[:, 0:128], mask)
                    pts.append((pt, trim))
                return pts

            def emit_av(qp, s, pts):
                norms = []
                for u in range(4):
                    jmax = 4 * qp + u
                    oe = pox.tile([128, HD + 1], f32, tag="oext", name="oe")
                    for j in range(jmax + 1):
                        pt, trim = pts[j]
                        c0 = u * 128 - trim
                        nc.tensor.matmul(
                            oe,
                            lhsT=pt[:, c0 : c0 + 128],
                            rhs=vext_sb[:, j * (HD + 1) : (j + 1) * (HD + 1)],
                            start=(j == 0),
                            stop=(j == jmax),
                        )
                    rc = smp.tile([128, 1], f32, tag="rc", name="rc")
                    nc.vector.reciprocal(rc, oe[:, HD : HD + 1])
                    on = onp.tile([128, 128], bf16, tag="on", name="on")
                    nc.vector.tensor_scalar_mul(on, oe[:, 0:HD], rc)
                    norms.append((on, s, qp * 512 + u * 128))
                return norms

            def emit_transposes(norms):
                for on, s, tq in norms:
                    tps = ptr.tile([128, 128], bf16, tag="tr", name="tps")
                    nc.tensor.transpose(tps, on, ident)
                    nc.vector.tensor_copy(
                        ohT_sb[:, s * T + tq : s * T + tq + 128], tps
                    )

            def emit_epilogue(qp):
                # output projection for this q-pass's 4 row tiles
                for u in range(4):
                    tt = qp * 4 + u
                    for half in range(2):
                        ot = osp.tile([128, EMB // 2], f32, tag="ostage", name="ot")
                        for jph in range(2):
                            jp = half * 2 + jph
                            ps = pmm.tile([128, 512], f32, tag="mm", name="ps")
                            for s in range(GS):
                                nc.tensor.matmul(
                                    ps,
                                    lhsT=ohT_sb[:, s * T + tt * 128 : s * T + (tt + 1) * 128],
                                    rhs=wp_sb[:, s * EMB + jp * 512 : s * EMB + (jp + 1) * 512],
                                    start=(s == 0),
                                    stop=(s == GS - 1),
                                )
                            nc.vector.tensor_copy(
                                ot[:, jph * 512 : (jph + 1) * 512], ps
                            )
                        eng = nc.sync if tt % 2 == 0 else nc.scalar
                        eng.dma_start(
                            out=out_d[tt, :, half * 1024 : (half + 1) * 1024], in_=ot
                        )

            pending = None  # (qp, s, pts) with scores emitted, AV not yet
            last_norms = []

            def advance(pending):
                # AV for the pending iteration, then the (lag-1) transposes of
                # the previous one; at a q-pass boundary flush and project
                nonlocal last_norms
                qp, s, pts = pending
                norms = emit_av(qp, s, pts)
                emit_transposes(last_norms)
                last_norms = norms
                if s == GS - 1:
                    emit_transposes(last_norms)
                    last_norms = []
                    emit_epilogue(qp)

            for qp in range(NQP):
                for s in range(GS):
                    pts = emit_scores(qp, s)
                    if pending is not None:
                        advance(pending)
                    pending = (qp, s, pts)
            advance(pending)

    nc.finalize()
    return nc


def _get_program():
    global _PROGRAM
    if _PROGRAM is None:
        _PROGRAM = _build_program()
    return _PROGRAM


def _pack(a, nchunk):
    """[nchunk*128, F] -> [128, nchunk*F] so it lands in SBUF layout with one
    contiguous DMA: out[p, c*F + f] = a[c*128 + p, f]."""
    n, f = a.shape
    assert n == nchunk * 128
    return np.ascontiguousarray(
        a.reshape(nchunk, 128, f).transpose(1, 0, 2).reshape(128, nchunk * f)
    )


def _make_in_maps(x, Wq, Wk, Wv, Wp):
    in_maps = []
    xTs = [_pack(np.asarray(x[b]).T.astype(_BF16), NE) for b in range(2)]
    for c in range(8):
        b, g = c // 4, c % 4
        sl = slice(g * GS * HD, (g + 1) * GS * HD)
        kv = slice(g * GS * HD, g * GS * HD + HD)
        in_maps.append(
            {
                "xTp": xTs[b],
                "wqp": _pack(np.asarray(Wq[sl, :]).T.astype(_BF16), NE),
                "wkp": _pack(np.asarray(Wk[kv, :]).T.astype(_BF16), NE),
                "wvp": _pack(np.asarray(Wv[kv, :]).T.astype(_BF16), NE),
                "wpp": _pack(np.asarray(Wp[:, sl]).T.astype(_BF16), GS),
            }
        )
    return in_maps


def run(x, Wq, Wk, Wv, Wp, bp, trace=False, **trace_kwargs):
    from concourse.bass_utils import run_bass_kernel_spmd

    nc = _get_program()
    in_maps = _make_in_maps(x, Wq, Wk, Wv, Wp)
    res = run_bass_kernel_spmd(
        nc, in_maps, core_ids=list(range(8)), trace=trace, **trace_kwargs
    )
    bp = np.asarray(bp, dtype=np.float32)
    y = np.empty((2, T, EMB), dtype=np.float32)
    for b in range(2):
        acc = res.results[4 * b]["partial"].copy()
        for g in range(1, 4):
            acc += res.results[4 * b + g]["partial"]
        y[b] = acc + bp
    return y, res


def kernel(x, Wq, Wk, Wv, Wp, bp):
    y, _ = run(x, Wq, Wk, Wv, Wp, bp, trace=False)
    return y


# revision 14
# speedup vs baseline: 1.1700x; 1.0209x over previous
"""GQA kernel for Trainium2, 8 NeuronCores.

Sharding: core c = (b, g) with b = c // 4 (batch), g = c % 4 (KV group).
Each core computes, for its batch b and group g (4 query heads, 1 KV head):
  qT[d, t] for the 4 heads, kT[d, t], v[t, d] projections (contraction over EMB,
  inputs pre-packed on host so EMB lands on SBUF partitions),
  causal flash-style attention in [k-part, q-free] score layout,
  and the partial output projection  partial_g = (attn out) @ Wp[:, g cols].T.
Host gathers: y[b] = sum_g partial[b, g] + bp.

All matmuls run in bf16 (fp32 PSUM accumulation); host pre-casts inputs.
Inputs are host-packed to the exact SBUF layout [128, free] so each tensor
loads with one contiguous DMA (dma_start issue cost dominates chunked loads).
Causal structure: scores for the diagonal 128-row k-blocks are trimmed to the
q-columns that can attend; only the first 128-col band of each diagonal block
needs an elementwise triangular mask.
"""

import numpy as np
import ml_dtypes

T = 2048
EMB = 2048
HD = 128
GS = 4          # query heads per core (per KV group)
NE = EMB // 128 # 16 contraction chunks
NT = T // 128   # 16 row tiles
NQP = T // 512  # 4 q passes of 512
SCALE = float(HD) ** -0.5

_BF16 = ml_dtypes.bfloat16
_PROGRAM = None


def _build_program():
    import concourse.bass as bass
    import concourse.tile as tile
    from concourse import bacc, mybir
    from concourse.masks import make_identity

    f32 = mybir.dt.float32
    bf16 = mybir.dt.bfloat16

    nc = bacc.Bacc("TRN2", target_bir_lowering=False, debug=False)

    # all inputs host-packed to [128 partitions, free] SBUF layout
    xT_d = nc.dram_tensor("xTp", [128, NE * T], bf16, kind="ExternalInput")
    wq_d = nc.dram_tensor("wqp", [128, NE * GS * HD], bf16, kind="ExternalInput")
    wk_d = nc.dram_tensor("wkp", [128, NE * HD], bf16, kind="ExternalInput")
    wv_d = nc.dram_tensor("wvp", [128, NE * HD], bf16, kind="ExternalInput")
    wp_d = nc.dram_tensor("wpp", [128, GS * EMB], bf16, kind="ExternalInput")
    out_d = nc.dram_tensor("partial", [T, EMB], f32, kind="ExternalOutput").rearrange(
        "(n p) m -> n p m", p=128
    )

    with tile.TileContext(nc) as tc:
        with (
            tc.tile_pool(name="big", bufs=1) as big,
            tc.tile_pool(name="pt", bufs=26) as ptp,
            tc.tile_pool(name="onorm", bufs=18) as onp,
            tc.tile_pool(name="ostage", bufs=3) as osp,
            tc.tile_pool(name="small", bufs=6) as smp,
            tc.tile_pool(name="mm", bufs=4, space="PSUM") as pmm,
            tc.tile_pool(name="oext", bufs=2, space="PSUM") as pox,
            tc.tile_pool(name="tr", bufs=2, space="PSUM") as ptr,
        ):
            xT_sb = big.tile([128, NE * T], bf16)
            wq_sb = big.tile([128, NE * GS * HD], bf16)
            wk_sb = big.tile([128, NE * HD], bf16)
            wv_sb = big.tile([128, NE * HD], bf16)
            wp_sb = big.tile([128, GS * EMB], bf16)
            qT_sb = big.tile([128, GS * T], bf16)
            kT_sb = big.tile([128, T], bf16)
            vT_sb = big.tile([128, T], bf16)
            vext_sb = big.tile([128, NT * (HD + 1)], bf16)
            ohT_sb = big.tile([128, GS * T], bf16)
            ident = big.tile([128, 128], bf16)
            mask = big.tile([128, 128], bf16)

            # constants: identity for PE transpose; triangular mask for the
            # first 128-col band of diagonal blocks (keep iff q_local >= k_local)
            make_identity(nc, ident)
            nc.gpsimd.memset(mask, 1.0)
            nc.gpsimd.affine_select(
                out=mask,
                in_=mask,
                compare_op=mybir.AluOpType.is_ge,
                fill=0.0,
                base=0,
                pattern=[[1, 128]],
                channel_multiplier=-1,
            )
            nc.vector.memset(vext_sb, 1.0)

            # input DMAs: contiguous block loads; xT quarters gate the
            # projection chains so they go first on the sync queue
            nc.scalar.dma_start(out=wk_sb, in_=wk_d[:, :])
            nc.scalar.dma_start(out=wv_sb, in_=wv_d[:, :])
            # wq/wp ride the same (sync) queue BEHIND all xT chunks: they are
            # not needed until the qT/output projections, and issuing them
            # early would steal HBM bandwidth from the arrival-critical xT
            for c in range(NE):
                nc.sync.dma_start(
                    out=xT_sb[:, c * T : (c + 1) * T], in_=xT_d[:, c * T : (c + 1) * T]
                )
            hw = NE * GS * HD // 2
            for q in range(2):
                nc.sync.dma_start(
                    out=wq_sb[:, q * hw : (q + 1) * hw],
                    in_=wq_d[:, q * hw : (q + 1) * hw],
                )
            nc.sync.dma_start(out=wp_sb, in_=wp_d[:, :])

            # PE clock warmup: ~24 back-to-back matmuls on garbage SBUF data
            # (nothing reads the result) so HAM reaches 2.4 GHz before the
            # first real, DMA-gated matmuls trickle in
            wgarb = big.tile([128, 512], bf16)
            nc.gpsimd.memset(wgarb, 1.0)
            wps = pmm.tile([128, 512], f32, tag="mm", name="wps")
            for _ in range(24):
                nc.tensor.matmul(wps, lhsT=ident, rhs=wgarb, start=True, stop=True)

            # kT + vT projections interleaved, chunk-outer so PE consumes each
            # xT chunk as it arrives. kT uses the 4 "mm" psum slots; vT borrows
            # the attention pools' slots (oext x2 + tr x2) so both run during
            # the DMA-arrival window.
            kss = [pmm.tile([128, 512], f32, tag="mm", name=f"kss{i}") for i in range(4)]
            vss = [
        pox.tile([128, 512], f32, tag="oext", name="vss0"),
        pox.tile([128, 512], f32, tag="oext", name="vss1"),
        ptr.tile([128, 512], f32, tag="tr", name="vss2"),
        ptr.tile([128, 512], f32, tag="tr", name="vss3"),
            ]
            for c in range(NE):
                for tp in range(4):
                    nc.tensor.matmul(
                        kss[tp],
                        lhsT=wk_sb[:, c * HD : (c + 1) * HD],
                        rhs=xT_sb[:, c * T + tp * 512 : c * T + (tp + 1) * 512],
                        start=(c == 0),
                        stop=(c == NE - 1),
                    )
                for tp in range(4):
                    nc.tensor.matmul(
                        vss[tp],
                        lhsT=wv_sb[:, c * HD : (c + 1) * HD],
                        rhs=xT_sb[:, c * T + tp * 512 : c * T + (tp + 1) * 512],
                        start=(c == 0),
                        stop=(c == NE - 1),
                    )
            for tp in range(4):
                nc.scalar.copy(kT_sb[:, tp * 512 : (tp + 1) * 512], kss[tp])
            for tp in range(4):
                nc.scalar.copy(vT_sb[:, tp * 512 : (tp + 1) * 512], vss[tp])
            for tt in range(NT):
                tv = ptr.tile([128, 128], bf16, tag="tr")
                nc.tensor.transpose(tv, vT_sb[:, tt * 128 : (tt + 1) * 128], ident)
                nc.vector.tensor_copy(
                    vext_sb[:, tt * (HD + 1) : tt * (HD + 1) + HD], tv
                )

            # qT projection per head: 4 parallel psum chains each
            for s in range(GS):
                pss = [pmm.tile([128, 512], f32, tag="mm", name=f"pss{i}") for i in range(4)]
                for c in range(NE):
                    for tp in range(4):
                        nc.tensor.matmul(
                            pss[tp],
                            lhsT=wq_sb[
                                :, c * GS * HD + s * HD : c * GS * HD + (s + 1) * HD
                            ],
                            rhs=xT_sb[:, c * T + tp * 512 : c * T + (tp + 1) * 512],
                            start=(c == 0),
                            stop=(c == NE - 1),
                        )
                for tp in range(4):
                    nc.scalar.copy(
                        qT_sb[:, s * T + tp * 512 : s * T + (tp + 1) * 512], pss[tp]
                    )

            # attention + output projection, software-pipelined: scores for
            # iteration i+1 are emitted before AV of iteration i so the PE
            # stream never waits for ACT's exp backlog at AV chain heads
            deferred = []

            def emit_scores(qp, s):
                pts = []
                for j in range(4 * qp + 4):
                    o = j - 4 * qp  # diagonal offset; <0 for full blocks
                    trim = 128 * o if o > 0 else 0
                    w = 512 - trim
                    q0 = s * T + qp * 512 + trim
                    ps = pmm.tile([128, w], f32, tag="mm", name="ps")
                    nc.tensor.matmul(
                        ps,
                        lhsT=kT_sb[:, j * 128 : (j + 1) * 128],
                        rhs=qT_sb[:, q0 : q0 + w],
                        start=True,
                        stop=True,
                    )
                    pt = ptp.tile([128, w], bf16, tag="pt", name="pt")
                    nc.scalar.activation(
                        pt, ps, mybir.ActivationFunctionType.Exp, scale=SCALE
                    )
                    if o >= 0:
                        # only the first 128-col band straddles the diagonal
                        nc.vector.tensor_mul(pt[:, 0:128], pt[:, 0:128], mask)
                    pts.append((pt, trim))
                return pts

            def emit_av(qp, s, pts):
                norms = []
                for u in range(4):
                    jmax = 4 * qp + u
                    oe = pox.tile([128, HD + 1], f32, tag="oext", name="oe")
                    for j in range(jmax + 1):
                        pt, trim = pts[j]
                        c0 = u * 128 - trim
                        nc.tensor.matmul(
                            oe,
                            lhsT=pt[:, c0 : c0 + 128],
                            rhs=vext_sb[:, j * (HD + 1) : (j + 1) * (HD + 1)],
                            start=(j == 0),
                            stop=(j == jmax),
                        )
                    rc = smp.tile([128, 1], f32, tag="rc", name="rc")
                    nc.vector.reciprocal(rc, oe[:, HD : HD + 1])
                    on = onp.tile([128, 128], bf16, tag="on", name="on")
                    nc.vector.tensor_scalar_mul(on, oe[:, 0:HD], rc)
                    norms.append((on, s, qp * 512 + u * 128))
                return norms

            def emit_transposes(norms):
                for on, s, tq in norms:
                    tps = ptr.tile([128, 128], bf16, tag="tr", name="tps")
                    nc.tensor.transpose(tps, on, ident)
                    nc.vector.tensor_copy(
                        ohT_sb[:, s * T + tq : s * T + tq + 128], tps
                    )

            def emit_epilogue(qp):
                # output projection for this q-pass's 4 row tiles
                for u in range(4):
                    tt = qp * 4 + u
                    for half in range(2):
                        ot = osp.tile([128, EMB // 2], f32, tag="ostage", name="ot")
                        for jph in range(2):
                            jp = half * 2 + jph
                            ps = pmm.tile([128, 512], f32, tag="mm", name="ps")
                            for s in range(GS):
                                nc.tensor.matmul(
                                    ps,
                                    lhsT=ohT_sb[:, s * T + tt * 128 : s * T + (tt + 1) * 128],
                                    rhs=wp_sb[:, s * EMB + jp * 512 : s * EMB + (jp + 1) * 512],
                                    start=(s == 0),
                                    stop=(s == GS - 1),
                                )
                            nc.vector.tensor_copy(
                                ot[:, jph * 512 : (jph + 1) * 512], ps
                            )
                        eng = nc.sync if tt % 2 == 0 else nc.scalar
                        eng.dma_start(
                            out=out_d[tt, :, half * 1024 : (half + 1) * 1024], in_=ot
                        )

            pending = None  # (qp, s, pts) with scores emitted, AV not yet
            last_norms = []

            def advance(pending):
                # AV for the pending iteration, then the (lag-1) transposes of
                # the previous one; at a q-pass boundary flush and project
                nonlocal last_norms
                qp, s, pts = pending
                norms = emit_av(qp, s, pts)
                emit_transposes(last_norms)
                last_norms = norms
                if s == GS - 1:
                    emit_transposes(last_norms)
                    last_norms = []
                    emit_epilogue(qp)

            for qp in range(NQP):
                for s in range(GS):
                    pts = emit_scores(qp, s)
                    if pending is not None:
                        advance(pending)
                    pending = (qp, s, pts)
            advance(pending)

    nc.finalize()
    return nc


def _get_program():
    global _PROGRAM
    if _PROGRAM is None:
        _PROGRAM = _build_program()
    return _PROGRAM


def _pack(a, nchunk):
    """[nchunk*128, F] -> [128, nchunk*F] so it lands in SBUF layout with one
    contiguous DMA: out[p, c*F + f] = a[c*128 + p, f]."""
    n, f = a.shape
    assert n == nchunk * 128
    return np.ascontiguousarray(
        a.reshape(nchunk, 128, f).transpose(1, 0, 2).reshape(128, nchunk * f)
    )


def _make_in_maps(x, Wq, Wk, Wv, Wp):
    in_maps = []
    xTs = [_pack(np.asarray(x[b]).T.astype(_BF16), NE) for b in range(2)]
    for c in range(8):
        b, g = c // 4, c % 4
        sl = slice(g * GS * HD, (g + 1) * GS * HD)
        kv = slice(g * GS * HD, g * GS * HD + HD)
        in_maps.append(
            {
                "xTp": xTs[b],
                "wqp": _pack(np.asarray(Wq[sl, :]).T.astype(_BF16), NE),
                "wkp": _pack(np.asarray(Wk[kv, :]).T.astype(_BF16), NE),
                "wvp": _pack(np.asarray(Wv[kv, :]).T.astype(_BF16), NE),
                "wpp": _pack(np.asarray(Wp[:, sl]).T.astype(_BF16), GS),
            }
        )
    return in_maps


def run(x, Wq, Wk, Wv, Wp, bp, trace=False, **trace_kwargs):
    from concourse.bass_utils import run_bass_kernel_spmd

    nc = _get_program()
    in_maps = _make_in_maps(x, Wq, Wk, Wv, Wp)
    res = run_bass_kernel_spmd(
        nc, in_maps, core_ids=list(range(8)), trace=trace, **trace_kwargs
    )
    bp = np.asarray(bp, dtype=np.float32)
    y = np.empty((2, T, EMB), dtype=np.float32)
    for b in range(2):
        acc = res.results[4 * b]["partial"].copy()
        for g in range(1, 4):
            acc += res.results[4 * b + g]["partial"]
        y[b] = acc + bp
    return y, res


def kernel(x, Wq, Wk, Wv, Wp, bp):
    y, _ = run(x, Wq, Wk, Wv, Wp, bp, trace=False)
    return y


# revision 15
# speedup vs baseline: 1.2414x; 1.0610x over previous
"""GQA kernel for Trainium2, 8 NeuronCores.

Sharding: core c = (b, g) with b = c // 4 (batch), g = c % 4 (KV group).
Each core computes, for its batch b and group g (4 query heads, 1 KV head):
  qT[d, t] for the 4 heads, kT[d, t], v[t, d] projections (contraction over EMB,
  inputs pre-packed on host so EMB lands on SBUF partitions),
  causal flash-style attention in [k-part, q-free] score layout,
  and the partial output projection  partial_g = (attn out) @ Wp[:, g cols].T.
Host gathers: y[b] = sum_g partial[b, g] + bp.

All matmuls run in bf16 (fp32 PSUM accumulation); host pre-casts inputs.
Inputs are host-packed to the exact SBUF layout [128, free] so each tensor
loads with one contiguous DMA (dma_start issue cost dominates chunked loads).
Causal structure: scores for the diagonal 128-row k-blocks are trimmed to the
q-columns that can attend; only the first 128-col band of each diagonal block
needs an elementwise triangular mask.
"""

import numpy as np
import ml_dtypes

T = 2048
EMB = 2048
HD = 128
GS = 4          # query heads per core (per KV group)
NE = EMB // 128 # 16 contraction chunks
NT = T // 128   # 16 row tiles
NQP = T // 512  # 4 q passes of 512
SCALE = float(HD) ** -0.5

_BF16 = ml_dtypes.bfloat16
_PROGRAM = None


def _build_program():
    import concourse.bass as bass
    import concourse.tile as tile
    from concourse import bacc, mybir
    from concourse.masks import make_identity

    f32 = mybir.dt.float32
    bf16 = mybir.dt.bfloat16

    nc = bacc.Bacc("TRN2", target_bir_lowering=False, debug=False)

    # all inputs host-packed to [128 partitions, free] SBUF layout
    xT_d = nc.dram_tensor("xTp", [128, NE * T], bf16, kind="ExternalInput")
    wq_d = nc.dram_tensor("wqp", [128, NE * GS * HD], bf16, kind="ExternalInput")
    wk_d = nc.dram_tensor("wkp", [128, NE * HD], bf16, kind="ExternalInput")
    wv_d = nc.dram_tensor("wvp", [128, NE * HD], bf16, kind="ExternalInput")
    wp_d = nc.dram_tensor("wpp", [128, GS * EMB], bf16, kind="ExternalInput")
    out_d = nc.dram_tensor("partial", [T, EMB], f32, kind="ExternalOutput").rearrange(
        "(n p) m -> n p m", p=128
    )

    with tile.TileContext(nc) as tc:
        with (
            tc.tile_pool(name="big", bufs=1) as big,
            tc.tile_pool(name="pt", bufs=40) as ptp,
            tc.tile_pool(name="onorm", bufs=18) as onp,
            tc.tile_pool(name="ostage", bufs=3) as osp,
            tc.tile_pool(name="small", bufs=6) as smp,
            tc.tile_pool(name="mm", bufs=4, space="PSUM") as pmm,
            tc.tile_pool(name="oext", bufs=2, space="PSUM") as pox,
            tc.tile_pool(name="tr", bufs=2, space="PSUM") as ptr,
        ):
            xT_sb = big.tile([128, NE * T], bf16)
            wq_sb = big.tile([128, NE * GS * HD], bf16)
            wk_sb = big.tile([128, NE * HD], bf16)
            wv_sb = big.tile([128, NE * HD], bf16)
            wp_sb = big.tile([128, GS * EMB], bf16)
            qT_sb = big.tile([128, GS * T], bf16)
            kT_sb = big.tile([128, T], bf16)
            vT_sb = big.tile([128, T], bf16)
            vext_sb = big.tile([128, NT * (HD + 1)], bf16)
            ohT_sb = big.tile([128, GS * T], bf16)
            ident = big.tile([128, 128], bf16)
            mask = big.tile([128, 128], bf16)

            # constants: identity for PE transpose; triangular mask for the
            # first 128-col band of diagonal blocks (keep iff q_local >= k_local)
            make_identity(nc, ident)
            nc.gpsimd.memset(mask, 1.0)
            nc.gpsimd.affine_select(
                out=mask,
                in_=mask,
                compare_op=mybir.AluOpType.is_ge,
                fill=0.0,
                base=0,
                pattern=[[1, 128]],
                channel_multiplier=-1,
            )
            nc.vector.memset(vext_sb, 1.0)

            # input DMAs: contiguous block loads; xT quarters gate the
            # projection chains so they go first on the sync queue
            nc.scalar.dma_start(out=wk_sb, in_=wk_d[:, :])
            nc.scalar.dma_start(out=wv_sb, in_=wv_d[:, :])
            # wq/wp ride the same (sync) queue BEHIND all xT chunks: they are
            # not needed until the qT/output projections, and issuing them
            # early would steal HBM bandwidth from the arrival-critical xT
            for c in range(NE):
                nc.sync.dma_start(
                    out=xT_sb[:, c * T : (c + 1) * T], in_=xT_d[:, c * T : (c + 1) * T]
                )
            hw = NE * GS * HD // 2
            for q in range(2):
                nc.sync.dma_start(
                    out=wq_sb[:, q * hw : (q + 1) * hw],
                    in_=wq_d[:, q * hw : (q + 1) * hw],
                )
            nc.sync.dma_start(out=wp_sb, in_=wp_d[:, :])

            # PE clock warmup: ~24 back-to-back matmuls on garbage SBUF data
            # (nothing reads the result) so HAM reaches 2.4 GHz before the
            # first real, DMA-gated matmuls trickle in
            wgarb = big.tile([128, 512], bf16)
            nc.gpsimd.memset(wgarb, 1.0)
            wps = pmm.tile([128, 512], f32, tag="mm", name="wps")
            for _ in range(24):
                nc.tensor.matmul(wps, lhsT=ident, rhs=wgarb, start=True, stop=True)

            # kT + vT projections interleaved, chunk-outer so PE consumes each
            # xT chunk as it arrives. kT uses the 4 "mm" psum slots; vT borrows
            # the attention pools' slots (oext x2 + tr x2) so both run during
            # the DMA-arrival window.
            kss = [pmm.tile([128, 512], f32, tag="mm", name=f"kss{i}") for i in range(4)]
            vss = [
        pox.tile([128, 512], f32, tag="oext", name="vss0"),
        pox.tile([128, 512], f32, tag="oext", name="vss1"),
        ptr.tile([128, 512], f32, tag="tr", name="vss2"),
        ptr.tile([128, 512], f32, tag="tr", name="vss3"),
            ]
            for c in range(NE):
                for tp in range(4):
                    nc.tensor.matmul(
                        kss[tp],
                        lhsT=wk_sb[:, c * HD : (c + 1) * HD],
                        rhs=xT_sb[:, c * T + tp * 512 : c * T + (tp + 1) * 512],
                        start=(c == 0),
                        stop=(c == NE - 1),
                    )
                for tp in range(4):
                    nc.tensor.matmul(
                        vss[tp],
                        lhsT=wv_sb[:, c * HD : (c + 1) * HD],
                        rhs=xT_sb[:, c * T + tp * 512 : c * T + (tp + 1) * 512],
                        start=(c == 0),
                        stop=(c == NE - 1),
                    )
            for tp in range(4):
                nc.scalar.copy(kT_sb[:, tp * 512 : (tp + 1) * 512], kss[tp])
            for tp in range(4):
                nc.scalar.copy(vT_sb[:, tp * 512 : (tp + 1) * 512], vss[tp])
            for tt in range(NT):
                tv = ptr.tile([128, 128], bf16, tag="tr")
                nc.tensor.transpose(tv, vT_sb[:, tt * 128 : (tt + 1) * 128], ident)
                nc.vector.tensor_copy(
                    vext_sb[:, tt * (HD + 1) : tt * (HD + 1) + HD], tv
                )

            # qT projection per head: 4 parallel psum chains each
            for s in range(GS):
                pss = [pmm.tile([128, 512], f32, tag="mm", name=f"pss{i}") for i in range(4)]
                for c in range(NE):
                    for tp in range(4):
                        nc.tensor.matmul(
                            pss[tp],
                            lhsT=wq_sb[
                                :, c * GS * HD + s * HD : c * GS * HD + (s + 1) * HD
                            ],
                            rhs=xT_sb[:, c * T + tp * 512 : c * T + (tp + 1) * 512],
                            start=(c == 0),
                            stop=(c == NE - 1),
                        )
                for tp in range(4):
                    nc.scalar.copy(
                        qT_sb[:, s * T + tp * 512 : s * T + (tp + 1) * 512], pss[tp]
                    )

            # attention + output projection, software-pipelined: scores for
            # iteration i+1 are emitted before AV of iteration i so the PE
            # stream never waits for ACT's exp backlog at AV chain heads
            deferred = []

            def emit_scores(qp, s):
                pts = []
                for j in range(4 * qp + 4):
                    o = j - 4 * qp  # diagonal offset; <0 for full blocks
                    trim = 128 * o if o > 0 else 0
                    w = 512 - trim
                    q0 = s * T + qp * 512 + trim
                    ps = pmm.tile([128, w], f32, tag="mm", name="ps")
                    nc.tensor.matmul(
                        ps,
                        lhsT=kT_sb[:, j * 128 : (j + 1) * 128],
                        rhs=qT_sb[:, q0 : q0 + w],
                        start=True,
                        stop=True,
                    )
                    pt = ptp.tile([128, w], bf16, tag="pt", name="pt")
                    nc.scalar.activation(
                        pt, ps, mybir.ActivationFunctionType.Exp, scale=SCALE
                    )
                    if o >= 0:
                        # only the first 128-col band straddles the diagonal
                        nc.vector.tensor_mul(pt[:, 0:128], pt[:, 0:128], mask)
                    pts.append((pt, trim))
                return pts

            def emit_av(qp, s, pts):
                norms = []
                for u in range(4):
                    jmax = 4 * qp + u
                    oe = pox.tile([128, HD + 1], f32, tag="oext", name="oe")
                    for j in range(jmax + 1):
                        pt, trim = pts[j]
                        c0 = u * 128 - trim
                        nc.tensor.matmul(
                            oe,
                            lhsT=pt[:, c0 : c0 + 128],
                            rhs=vext_sb[:, j * (HD + 1) : (j + 1) * (HD + 1)],
                            start=(j == 0),
                            stop=(j == jmax),
                        )
                    rc = smp.tile([128, 1], f32, tag="rc", name="rc")
                    nc.vector.reciprocal(rc, oe[:, HD : HD + 1])
                    on = onp.tile([128, 128], bf16, tag="on", name="on")
                    nc.vector.tensor_scalar_mul(on, oe[:, 0:HD], rc)
                    norms.append((on, s, qp * 512 + u * 128))
                return norms

            def emit_transposes(norms):
                for on, s, tq in norms:
                    tps = ptr.tile([128, 128], bf16, tag="tr", name="tps")
                    nc.tensor.transpose(tps, on, ident)
                    nc.vector.tensor_copy(
                        ohT_sb[:, s * T + tq : s * T + tq + 128], tps
                    )

            def emit_epilogue(qp):
                # output projection for this q-pass's 4 row tiles
                for u in range(4):
                    tt = qp * 4 + u
                    for half in range(2):
                        ot = osp.tile([128, EMB // 2], f32, tag="ostage", name="ot")
                        for jph in range(2):
                            jp = half * 2 + jph
                            ps = pox.tile([128, 512], f32, tag="oext", name="ps")
                            for s in range(GS):
                                nc.tensor.matmul(
                                    ps,
                                    lhsT=ohT_sb[:, s * T + tt * 128 : s * T + (tt + 1) * 128],
                                    rhs=wp_sb[:, s * EMB + jp * 512 : s * EMB + (jp + 1) * 512],
                                    start=(s == 0),
                                    stop=(s == GS - 1),
                                )
                            nc.vector.tensor_copy(
                                ot[:, jph * 512 : (jph + 1) * 512], ps
                            )
                        eng = nc.sync if tt % 2 == 0 else nc.scalar
                        eng.dma_start(
                            out=out_d[tt, :, half * 1024 : (half + 1) * 1024], in_=ot
                        )

            last_norms = []

            def advance(pending):
                # AV for the pending iteration, then the (lag-1) transposes of
                # the previous one; at a q-pass boundary flush and project
                nonlocal last_norms
                qp, s, pts = pending
                norms = emit_av(qp, s, pts)
                emit_transposes(last_norms)
                last_norms = norms
                if s == GS - 1:
                    emit_transposes(last_norms)
                    last_norms = []
                    emit_epilogue(qp)

            window = []
            for qp in range(NQP):
                for s in range(GS):
                    pts = emit_scores(qp, s)
                    window.append((qp, s, pts))
                    if len(window) > 2:
                        advance(window.pop(0))
            for w in window:
                advance(w)

    nc.finalize()
    return nc


def _get_program():
    global _PROGRAM
    if _PROGRAM is None:
        _PROGRAM = _build_program()
    return _PROGRAM


def _pack(a, nchunk):
    """[nchunk*128, F] -> [128, nchunk*F] so it lands in SBUF layout with one
    contiguous DMA: out[p, c*F + f] = a[c*128 + p, f]."""
    n, f = a.shape
    assert n == nchunk * 128
    return np.ascontiguousarray(
        a.reshape(nchunk, 128, f).transpose(1, 0, 2).reshape(128, nchunk * f)
    )


def _make_in_maps(x, Wq, Wk, Wv, Wp):
    in_maps = []
    xTs = [_pack(np.asarray(x[b]).T.astype(_BF16), NE) for b in range(2)]
    for c in range(8):
        b, g = c // 4, c % 4
        sl = slice(g * GS * HD, (g + 1) * GS * HD)
        kv = slice(g * GS * HD, g * GS * HD + HD)
        in_maps.append(
            {
                "xTp": xTs[b],
                "wqp": _pack(np.asarray(Wq[sl, :]).T.astype(_BF16), NE),
                "wkp": _pack(np.asarray(Wk[kv, :]).T.astype(_BF16), NE),
                "wvp": _pack(np.asarray(Wv[kv, :]).T.astype(_BF16), NE),
                "wpp": _pack(np.asarray(Wp[:, sl]).T.astype(_BF16), GS),
            }
        )
    return in_maps


def run(x, Wq, Wk, Wv, Wp, bp, trace=False, **trace_kwargs):
    from concourse.bass_utils import run_bass_kernel_spmd

    nc = _get_program()
    in_maps = _make_in_maps(x, Wq, Wk, Wv, Wp)
    res = run_bass_kernel_spmd(
        nc, in_maps, core_ids=list(range(8)), trace=trace, **trace_kwargs
    )
    bp = np.asarray(bp, dtype=np.float32)
    y = np.empty((2, T, EMB), dtype=np.float32)
    for b in range(2):
        acc = res.results[4 * b]["partial"].copy()
        for g in range(1, 4):
            acc += res.results[4 * b + g]["partial"]
        y[b] = acc + bp
    return y, res


def kernel(x, Wq, Wk, Wv, Wp, bp):
    y, _ = run(x, Wq, Wk, Wv, Wp, bp, trace=False)
    return y
